# revision 12
# baseline (speedup 1.0000x reference)
"""Trainium2 Bass kernel for conv-QKV cosine-sim attention block.

Module: 3x3 conv -> qkv -> cosine-sim attention (smoothed) with per-head
Gaussian positional bias -> 1x1 conv -> BatchNorm(train stats) -> ReLU.

Sharding: data-parallel over batch (B=8, one image per core). Cross-core
AllReduce (512 floats) for the BatchNorm batch statistics.

Math restructuring (all on-device unless noted):
- conv3x3 as 9 shifted matmuls over a zero-padded SBUF image.
- attention computed transposed: attnT[m,n] = sum_d k[d,m] q[d,n].
- cosine-sim smooth folded: attn/(qn*kn + 1e-4) ~= attn/(qn*kn) with
  rel. error ~1.6e-6 (qn*kn ~ 64 >> 1e-4).
- positional bias pb = exp(-sf*dis)/rowsum factorizes: rowsum is a
  Kronecker product of 32x32 row-sums (rA ox rB).
- kninv folded into pb via exp(-sf*dis + ln(kninv)); qninv*rowsum_inv
  applied on the attention output's free axis.
- Host passes only layout-rearranged inputs and input-independent
  constants (dis/dyy/dxx grids).
"""

import os
import sys

import numpy as np

for _p in ("/opt/trn_rl_repo", "/root/.axon_site/_ro/trn_rl_repo"):
    if os.path.isdir(_p) and _p not in sys.path:
        sys.path.insert(0, _p)

import ml_dtypes  # noqa: E402

import concourse.bass as bass  # noqa: E402
import concourse.mybir as mybir  # noqa: E402
import concourse.tile as tile  # noqa: E402
from concourse import bacc  # noqa: E402
from concourse.bass_utils import run_bass_kernel_spmd  # noqa: E402

F32 = mybir.dt.float32
BF16 = mybir.dt.bfloat16
AF = mybir.ActivationFunctionType
ALU = mybir.AluOpType
AX = mybir.AxisListType

B, C, H, W = 8, 256, 32, 32
HEADS, D = 8, 64
N = H * W           # 1024
INNER = HEADS * D   # 512
NCORES = 8
SMOOTH = 1e-4
BN_EPS = 1e-5

_CACHE = {}


def _register_const(nc, value, dtype=F32):
    t = nc.alloc_sbuf_tensor(f"const-{dtype.name}-{value}", [128, 1], dtype)
    nc.gpsimd.memset(t.ap(), value)
    nc.const_aps.aps[(dtype, value)] = t.ap()


def _build_bass():
    nc = bacc.Bacc(num_devices=NCORES)
    _register_const(nc, SMOOTH)
    _register_const(nc, BN_EPS)
    _register_const(nc, -0.5)
    _register_const(nc, -1.0)
    nc.all_engine_barrier()

    # ---- kernel I/O ----
    x_d = nc.dram_tensor("x", [2, 128, N], F32, kind="ExternalInput")
    w_d = nc.dram_tensor("w", [9, C, 3 * INNER], F32, kind="ExternalInput")
    wo_d = nc.dram_tensor("wo", [INNER, C], F32, kind="ExternalInput")
    dis_d = nc.dram_tensor("dis", [N, N], BF16, kind="ExternalInput")
    dyy_d = nc.dram_tensor("dyy", [32, 32], F32, kind="ExternalInput")
    dxx_d = nc.dram_tensor("dxx", [32, 32], F32, kind="ExternalInput")
    hs_d = nc.dram_tensor("hs", [1, HEADS], F32, kind="ExternalInput")
    gamma_d = nc.dram_tensor("gamma", [128, 2], F32, kind="ExternalInput")
    beta_d = nc.dram_tensor("beta", [128, 2], F32, kind="ExternalInput")
    y_d = nc.dram_tensor("y", [C, N], F32, kind="ExternalOutput")

    # ---- internal DRAM (bounce + collective) ----
    nsf_b = nc.dram_tensor("nsf_b", [HEADS], F32)
    lnk_b = [nc.dram_tensor(f"lnk_b{h}", [N], F32) for h in range(HEADS)]
    combo_b = [nc.dram_tensor(f"combo_b{h}", [N], F32) for h in range(HEADS)]
    rab_b = [nc.dram_tensor(f"rab_b{h}", [64], F32) for h in range(HEADS)]
    ar_in = nc.dram_tensor("ar_in", [128, 4], F32)
    ar_out = nc.dram_tensor("ar_out", [128, 4], F32, addr_space="Shared")

    with tile.TileContext(nc) as tc:
        with (
            tc.tile_pool(name="singles", bufs=1) as singles,
            tc.tile_pool(name="wstg", bufs=2) as wstg,
            tc.tile_pool(name="wbf", bufs=3) as wbfp,
            tc.tile_pool(name="qkv", bufs=2) as qkvp,
            tc.tile_pool(name="pbp", bufs=4) as pbp,
            tc.tile_pool(name="spool", bufs=1) as spool,
            tc.tile_pool(name="work", bufs=2) as work,
            tc.tile_pool(name="rows", bufs=1) as rows,
            tc.tile_pool(name="small", bufs=4) as small,
            tc.tile_pool(name="outs", bufs=1) as outsp,
            tc.tile_pool(name="ps1", bufs=2, space="PSUM") as ps1,
            tc.tile_pool(name="ps2", bufs=2, space="PSUM") as ps2,
            tc.tile_pool(name="ps3", bufs=2, space="PSUM") as ps3,
        ):
            # ================= setup =================
            # head scales: s = sigmoid(hs)*(0.4-0.003)+0.003 ; sf = 1/(2 s^2)
            hs_sb = small.tile([1, HEADS], F32, name="hs_sb")
            nc.sync.dma_start(out=hs_sb, in_=hs_d[:])
            sig = small.tile([1, HEADS], F32, name="sig")
            nc.scalar.activation(out=sig, in_=hs_sb, func=AF.Exp, scale=-1.0)
            nc.vector.tensor_scalar_add(sig, sig, 1.0)
            nc.vector.reciprocal(out=sig, in_=sig)  # sigmoid(hs)
            s_t = small.tile([1, HEADS], F32, name="s_t")
            nc.vector.tensor_scalar(
                out=s_t, in0=sig, scalar1=0.397, scalar2=0.003,
                op0=ALU.mult, op1=ALU.add,
            )
            sinv = small.tile([1, HEADS], F32, name="sinv")
            nc.vector.reciprocal(out=sinv, in_=s_t)
            nsf = small.tile([1, HEADS], F32, name="nsf")
            nc.vector.tensor_tensor(nsf, sinv, sinv, ALU.mult)
            nc.vector.tensor_scalar_mul(nsf, nsf, -0.5)  # -1/(2 s^2)
            nc.sync.dma_start(out=nsf_b[:], in_=nsf[0:1, :])
            nsf_bc = singles.tile([128, HEADS], F32, name="nsf_bc")
            nsf_bcast_ap = bass.AP(
                tensor=nsf_b, offset=0, ap=[[0, 128], [1, HEADS]]
            )
            nc.gpsimd.dma_start(out=nsf_bc, in_=nsf_bcast_ap)

            # distance grids
            dis_sb = singles.tile([128, 8, N], BF16, name="dis_sb")
            nc.sync.dma_start(
                out=dis_sb, in_=dis_d[:].rearrange("(mt p) n -> p mt n", p=128)
            )
            dyy_sb = small.tile([32, 32], F32, name="dyy_sb")
            nc.sync.dma_start(out=dyy_sb, in_=dyy_d[:])
            dxx_sb = small.tile([32, 32], F32, name="dxx_sb")
            nc.sync.dma_start(out=dxx_sb, in_=dxx_d[:])

            # gamma / beta
            gam_sb = small.tile([128, 2], F32, name="gam_sb")
            nc.sync.dma_start(out=gam_sb, in_=gamma_d[:])
            bet_sb = small.tile([128, 2], F32, name="bet_sb")
            nc.sync.dma_start(out=bet_sb, in_=beta_d[:])

            # x -> padded bf16 image [128, ic_chunk, 34, 34]
            xpad = singles.tile([128, 2, 34, 34], BF16, name="xpad")
            nc.vector.memset(xpad, 0.0)
            with tc.tile_pool(name="xstg", bufs=1) as xstgp:
                x_stg = xstgp.tile([128, 2, N], F32, name="x_stg")
                nc.sync.dma_start(
                    out=x_stg, in_=x_d[:].rearrange("c p n -> p c n")
                )
                for c in range(2):
                    nc.vector.tensor_copy(
                        out=xpad[:, c, 1:33, 1:33],
                        in_=x_stg[:, c, :].rearrange("p (h w) -> p h w", h=32),
                    )

            # ones column for partition-sum matmuls
            ones_bf = singles.tile([128, 1], BF16, name="ones_bf")
            nc.vector.memset(ones_bf, 1.0)

            # attention output (proj rhs): [128, kc, N] bf16
            attout = outsp.tile([128, 4, N], BF16, name="attout")

            # ---------- helpers ----------
            def conv_pass(ot, dst_bf):
                """qkv conv for output-channel tile ot -> dst_bf [128, N] bf16."""
                wt_bf = wbfp.tile([128, 2, 9, 128], BF16, tag="wt_bf")
                for c in range(2):
                    # w[t, c*128+p, ot*128+oc] -> [p, t, oc]
                    wt_stg = wstg.tile([128, 9, 128], F32, tag="wt_stg")
                    nc.sync.dma_start(
                        out=wt_stg,
                        in_=w_d[:, c * 128:(c + 1) * 128, ot * 128:(ot + 1) * 128]
                        .rearrange("t p o -> p t o"),
                    )
                    nc.vector.tensor_copy(out=wt_bf[:, c], in_=wt_stg)
                for nch in range(2):
                    ps = ps1.tile([128, 512], F32, tag="pc", name=f"pc_{ot}_{nch}")
                    first = True
                    for c in range(2):
                        for t in range(9):
                            dy, dx = t // 3, t % 3
                            r0 = nch * 16 + dy
                            nc.tensor.matmul(
                                ps,
                                lhsT=wt_bf[:, c, t, :],
                                rhs=xpad[:, c, r0:r0 + 16, dx:dx + 32],
                                start=first,
                                stop=(c == 1 and t == 8),
                            )
                            first = False
                    nc.any.tensor_copy(
                        out=dst_bf[:, nch * 512:(nch + 1) * 512], in_=ps
                    )

            def head_norms(j, q_bf, k_bf):
                """Per-head normalizers for pair j (heads 2j, 2j+1).

                Returns (lnk_col, combo_bc): lnk_col [128, 8] per-head ln(kninv)
                columns indexed [p, mt] per head; combo_bc [128, N] with rows
                0:64 = qninv*rinv of even head, 64:128 odd head.
                """
                res_lnk = []
                for h_i, (base, hd) in enumerate(((0, 2 * j), (64, 2 * j + 1))):
                    qsq_bf = work.tile([128, N], BF16, tag="qsq_bf")
                    nc.vector.tensor_tensor(
                        qsq_bf[base:base + 64], q_bf[base:base + 64],
                        q_bf[base:base + 64], ALU.mult,
                    )
                    ksq_bf = work.tile([128, N], BF16, tag="ksq_bf")
                    nc.vector.tensor_tensor(
                        ksq_bf[base:base + 64], k_bf[base:base + 64],
                        k_bf[base:base + 64], ALU.mult,
                    )
                    qn_row = rows.tile([1, N], F32, tag="qn_row")
                    lnk_row = rows.tile([1, N], F32, tag="lnk_row")
                    for nch in range(2):
                        pq = ps2.tile([1, 512], F32, tag="pqn")
                        nc.tensor.matmul(
                            pq,
                            lhsT=ones_bf[base:base + 64, :],
                            rhs=qsq_bf[base:base + 64,
                                       nch * 512:(nch + 1) * 512],
                            start=True, stop=True,
                        )
                        # qninv = 1/sqrt(qn2 + SMOOTH) = exp(-0.5*ln(.))
                        lnq = rows.tile([1, 512], F32, tag="lnq")
                        nc.scalar.activation(
                            out=lnq, in_=pq, func=AF.Ln, bias=SMOOTH,
                        )
                        nc.scalar.activation(
                            out=qn_row[0:1, nch * 512:(nch + 1) * 512],
                            in_=lnq, func=AF.Exp, scale=-0.5,
                        )
                        pk = ps2.tile([1, 512], F32, tag="pqn")
                        nc.tensor.matmul(
                            pk,
                            lhsT=ones_bf[base:base + 64, :],
                            rhs=ksq_bf[base:base + 64,
                                       nch * 512:(nch + 1) * 512],
                            start=True, stop=True,
                        )
                        # ln(kninv) = -0.5*ln(kn2 + SMOOTH)
                        nc.scalar.activation(
                            out=lnk_row[0:1, nch * 512:(nch + 1) * 512],
                            in_=pk, func=AF.Ln, bias=SMOOTH,
                        )

                    nc.vector.tensor_scalar_mul(lnk_row, lnk_row, -0.5)
                    nc.sync.dma_start(out=lnk_b[hd][:], in_=lnk_row[0:1, :])
                    lnk_col = small.tile([128, 8], F32, tag="lnk_col")
                    nc.sync.dma_start(
                        out=lnk_col,
                        in_=lnk_b[hd][:].rearrange("(mt p) -> p mt", p=128),
                    )
                    res_lnk.append(lnk_col)

                    # positional-bias row-sum via Kronecker factors
                    eA = small.tile([32, 32], F32, tag="eA")
                    nc.scalar.activation(
                        out=eA, in_=dyy_sb, func=AF.Exp,
                        scale=nsf_bc[0:32, hd:hd + 1],
                    )
                    eB = small.tile([32, 32], F32, tag="eB")
                    nc.scalar.activation(
                        out=eB, in_=dxx_sb, func=AF.Exp,
                        scale=nsf_bc[0:32, hd:hd + 1],
                    )
                    rA = small.tile([32, 1], F32, tag="rA")
                    nc.vector.reduce_sum(rA, eA, axis=AX.X)
                    rB = small.tile([32, 1], F32, tag="rB")
                    nc.vector.reduce_sum(rB, eB, axis=AX.X)
                    nc.sync.dma_start(out=rab_b[hd][0:32], in_=rA[:, 0:1])
                    nc.sync.dma_start(out=rab_b[hd][32:64], in_=rB[:, 0:1])
                    rab_row = small.tile([1, 64], F32, tag="rab_row")
                    nc.sync.dma_start(out=rab_row[0:1, :], in_=rab_b[hd][None, :])
                    r_row = rows.tile([1, 32, 32], F32, tag="r_row")
                    nc.vector.tensor_tensor(
                        r_row,
                        rab_row[:, 0:32, None].to_broadcast((1, 32, 32)),
                        rab_row[:, None, 32:64].to_broadcast((1, 32, 32)),
                        ALU.mult,
                    )
                    r_flat = r_row.rearrange("a b c -> a (b c)")
                    nc.vector.reciprocal(out=r_flat, in_=r_flat)
                    nc.vector.tensor_tensor(qn_row, qn_row, r_flat, ALU.mult)
                    nc.sync.dma_start(out=combo_b[hd][:], in_=qn_row[0:1, :])

                combo_bc = work.tile([128, N], BF16, tag="combo_bc")
                for h_i, base in enumerate((0, 64)):
                    hd = 2 * j + h_i
                    bc_ap = bass.AP(
                        tensor=combo_b[hd], offset=0, ap=[[0, 64], [1, N]]
                    )
                    nc.gpsimd.dma_start(out=combo_bc[base:base + 64, :], in_=bc_ap)
                return res_lnk, combo_bc

            def pair_attention(j, q_bf, k_bf, vt_bf, lnks, combo_bc):
                """Attention for heads (2j, 2j+1) -> attout[:, j, :]."""
                s_tiles = []
                for h_i, base in enumerate((0, 64)):
                    hd = 2 * j + h_i
                    lnk_col = lnks[h_i]
                    s_bf = spool.tile([128, 8, N], BF16, tag=f"s_bf{h_i}")
                    for mt in range(8):
                        pb_bf = pbp.tile([128, N], BF16, tag="pb_bf")
                        nc.scalar.activation(
                            out=pb_bf, in_=dis_sb[:, mt, :],
                            func=AF.Exp,
                            scale=nsf_bc[:, hd:hd + 1],
                            bias=lnk_col[:, mt:mt + 1],
                        )
                        for nch in range(2):
                            pa = ps2.tile([128, 512], F32, tag="pa")
                            nc.tensor.matmul(
                                pa,
                                lhsT=k_bf[base:base + 64,
                                          mt * 128:(mt + 1) * 128],
                                rhs=q_bf[base:base + 64,
                                         nch * 512:(nch + 1) * 512],
                                start=True, stop=True,
                            )
                            nc.vector.tensor_tensor(
                                s_bf[:, mt, nch * 512:(nch + 1) * 512],
                                pa,
                                pb_bf[:, nch * 512:(nch + 1) * 512],
                                ALU.mult,
                            )
                    s_tiles.append(s_bf)

                for h_i, base in enumerate((0, 64)):
                    s_bf = s_tiles[h_i]
                    for nch in range(2):
                        pv = ps3.tile([128, 512], F32, tag="psv")
                        for mc in range(8):
                            nc.tensor.matmul(
                                pv,
                                lhsT=vt_bf[:, mc, :],
                                rhs=s_bf[:, mc, nch * 512:(nch + 1) * 512],
                                start=(mc == 0), stop=(mc == 7),
                            )
                        nc.vector.tensor_tensor(
                            attout[base:base + 64, j,
                                   nch * 512:(nch + 1) * 512],
                            pv[base:base + 64, :],
                            combo_bc[base:base + 64,
                                     nch * 512:(nch + 1) * 512],
                            ALU.mult,
                        )

            # ================= main flow =================
            for j in range(4):
                q_bf = qkvp.tile([128, N], BF16, tag="q_bf")
                k_bf = qkvp.tile([128, N], BF16, tag="k_bf")
                v_bf = qkvp.tile([128, N], BF16, tag="v_bf")
                conv_pass(j, q_bf)
                conv_pass(4 + j, k_bf)
                conv_pass(8 + j, v_bf)
                vt_bf = qkvp.tile([128, 8, 128], BF16, tag="vt_bf")
                for mc in range(8):
                    nc.sync.dma_start_transpose(
                        vt_bf[:, mc, :], v_bf[:, mc * 128:(mc + 1) * 128]
                    )
                lnks, combo_bc = head_norms(j, q_bf, k_bf)
                pair_attention(j, q_bf, k_bf, vt_bf, lnks, combo_bc)

            # ---- output projection (1x1 conv) ----
            wo_stg = wstg.tile([128, 4, C], F32, tag="wo_stg")
            nc.sync.dma_start(
                out=wo_stg, in_=wo_d[:].rearrange("(kc p) c -> p kc c", p=128)
            )
            wo_bf = wbfp.tile([128, 4, C], BF16, tag="wo_bf")
            nc.vector.tensor_copy(out=wo_bf, in_=wo_stg)

            proj = outsp.tile([128, 2, N], F32, name="proj")
            y_sb = outsp.tile([128, 2, N], F32, name="y_sb")
            stats = small.tile([128, 4], F32, name="stats")
            for ct in range(2):
                for nch in range(2):
                    pp = ps1.tile([128, 512], F32, tag="pc", name=f"pp_{ct}_{nch}")
                    for kc in range(4):
                        nc.tensor.matmul(
                            pp,
                            lhsT=wo_bf[:, kc, ct * 128:(ct + 1) * 128],
                            rhs=attout[:, kc, nch * 512:(nch + 1) * 512],
                            start=(kc == 0), stop=(kc == 3),
                        )
                    nc.any.tensor_copy(
                        out=proj[:, ct, nch * 512:(nch + 1) * 512], in_=pp
                    )
                nc.vector.reduce_sum(
                    stats[:, 2 * ct:2 * ct + 1], proj[:, ct, :], axis=AX.X
                )
                nc.scalar.activation(
                    out=y_sb[:, ct, :], in_=proj[:, ct, :], func=AF.Square,
                    accum_out=stats[:, 2 * ct + 1:2 * ct + 2],
                )

            # ---- BatchNorm stats AllReduce ----
            nc.sync.dma_start(out=ar_in[:], in_=stats)
            if os.environ.get("NO_COLLECTIVE", "0") != "1":
                nc.gpsimd.collective_compute(
                    "AllReduce",
                    ALU.add,
                    replica_groups=[list(range(NCORES))],
                    ins=[ar_in[:]],
                    outs=[ar_out[:]],
                )
            else:
                nc.sync.dma_start(out=ar_out[:], in_=stats)
            stats_g = small.tile([128, 4], F32, name="stats_g")
            nc.sync.dma_start(out=stats_g, in_=ar_out[:])

            mean = small.tile([128, 2], F32, name="mean")
            var = small.tile([128, 2], F32, name="var")
            scl = small.tile([128, 2], F32, name="scl")
            shf = small.tile([128, 2], F32, name="shf")
            cnt_inv = 1.0 / (B * N)
            if os.environ.get("NO_COLLECTIVE", "0") == "1":
                cnt_inv = 1.0 / N
            for ct in range(2):
                nc.vector.tensor_scalar_mul(
                    mean[:, ct:ct + 1], stats_g[:, 2 * ct:2 * ct + 1], cnt_inv
                )
            msq = small.tile([128, 2], F32, name="msq")
            nc.vector.tensor_tensor(msq, mean, mean, ALU.mult)
            for ct in range(2):
                nc.vector.scalar_tensor_tensor(
                    out=var[:, ct:ct + 1],
                    in0=stats_g[:, 2 * ct + 1:2 * ct + 2],
                    scalar=cnt_inv,
                    in1=msq[:, ct:ct + 1],
                    op0=ALU.mult,
                    op1=ALU.subtract,
                )
            rstd = small.tile([128, 2], F32, name="rstd")
            lnv = small.tile([128, 2], F32, name="lnv")
            nc.scalar.activation(out=lnv, in_=var, func=AF.Ln, bias=BN_EPS)
            nc.scalar.activation(out=rstd, in_=lnv, func=AF.Exp, scale=-0.5)
            nc.vector.tensor_tensor(scl, gam_sb, rstd, ALU.mult)
            nc.vector.scalar_tensor_tensor(
                out=shf, in0=mean, scalar=-1.0, in1=scl,
                op0=ALU.mult, op1=ALU.mult,
            )
            nc.vector.tensor_add(shf, shf, bet_sb)

            for ct in range(2):
                nc.scalar.activation(
                    out=y_sb[:, ct, :], in_=proj[:, ct, :], func=AF.Relu,
                    scale=scl[:, ct:ct + 1], bias=shf[:, ct:ct + 1],
                )
            nc.sync.dma_start(
                out=y_d[:].rearrange("(ct p) n -> p ct n", p=128), in_=y_sb
            )

    nc.finalize()
    return nc


def _host_inputs(x, w_qkv, head_scale, w_out, gamma, beta):
    """Build per-core input maps (layout rearrangement only)."""
    # conv weights: [oc, ic, ky, kx] -> [t, ic, oc]
    w_t = np.ascontiguousarray(
        w_qkv.reshape(3 * INNER, C, 9).transpose(2, 1, 0)
    ).astype(np.float32)
    wo_t = np.ascontiguousarray(w_out[:, :, 0, 0].T).astype(np.float32)

    yy, xx = np.meshgrid(np.arange(H), np.arange(W), indexing="ij")
    yf = (yy.reshape(-1) / H).astype(np.float32)
    xf = (xx.reshape(-1) / W).astype(np.float32)
    dy = yf[:, None] - yf[None, :]
    dx = xf[:, None] - xf[None, :]
    dis = (dy * dy + dx * dx).astype(np.float32)          # [N, N]
    ys = (np.arange(32, dtype=np.float32) / H)
    xs = (np.arange(32, dtype=np.float32) / W)
    dyy = (ys[:, None] - ys[None, :]) ** 2                # [32, 32]
    dxx = (xs[:, None] - xs[None, :]) ** 2

    common = {
        "w": w_t,
        "wo": wo_t,
        "dis": dis.astype(ml_dtypes.bfloat16),
        "dyy": dyy.astype(np.float32),
        "dxx": dxx.astype(np.float32),
        "hs": head_scale.reshape(1, HEADS).astype(np.float32),
        "gamma": np.ascontiguousarray(
            gamma.reshape(2, 128).T).astype(np.float32),
        "beta": np.ascontiguousarray(
            beta.reshape(2, 128).T).astype(np.float32),
    }
    in_maps = []
    for b in range(B):
        m = dict(common)
        m["x"] = np.ascontiguousarray(
            x[b].reshape(2, 128, N)).astype(np.float32)
        in_maps.append(m)
    return in_maps


def _run(inputs, trace=False):
    if "nc" not in _CACHE:
        _CACHE["nc"] = _build_bass()
    nc = _CACHE["nc"]
    in_maps = _host_inputs(**inputs)
    res = run_bass_kernel_spmd(
        nc, in_maps, core_ids=list(range(NCORES)), trace=trace
    )
    y = np.stack([res.results[i]["y"] for i in range(NCORES)])
    return y.reshape(B, C, H, W).astype(np.float32), res


def kernel(**inputs):
    y, _ = _run(inputs, trace=False)
    return y
